# revision 6
# baseline (speedup 1.0000x reference)
"""Trainium2 Bass kernel for sparse autoencoder (topk masking).

  encoder:  pre = x @ W_enc.T + b_enc ; enc = relu(pre)
  topk:     per-row top-32 of enc kept, rest zeroed  -> encoded_sparse
  decoder:  dec = encoded_sparse @ W_dec.T + b_dec
  returns (encoded_sparse, dec)

Sharding: pure data-parallel over the batch dim across 8 NeuronCores
(1024 rows per core).  No collectives needed.

Per-core pipeline (v1):
  Phase A: stream W_encT chunks once (h-outer loop), fp32 matmuls into
           PSUM, fused ReLU on ScalarE, park dense `enc` in a DRAM
           scratch tile.
  Phase B: per 128-row tile: 4 rounds of (max8 + match_replace8) on
           VectorE -> exact top-32 removal; encoded_sparse = enc - work;
           PE-transpose the masked tile to bf16 encT for the decoder.
  Phase C: dense bf16 decode, h-outer streaming of W_decT, PSUM
           accumulation over 128 h-chunks, bias via K=1 matmul.
"""

import sys

sys.path.insert(0, "/opt/trn_rl_repo")

import numpy as np
import ml_dtypes

B, D, H, O, K = 8192, 1024, 16384, 1024, 32
NCORES = 8
BSH = B // NCORES  # 1024 rows per core
NBT = BSH // 128  # 8 row-tiles of 128
HCH = 512  # encoder h-chunk
NHC = H // HCH  # 32
NDC = D // 128  # 8 contraction chunks

TRACE = False
LAST_RESULTS = {}

_cache = {}


def _build():
    import concourse.bass as bass  # noqa: F401
    import concourse.mybir as mybir
    import concourse.tile as tile
    from concourse import bacc
    from concourse.masks import make_identity
    from contextlib import ExitStack

    fp32 = mybir.dt.float32
    bf16 = mybir.dt.bfloat16
    RELU = mybir.ActivationFunctionType.Relu
    COPY = mybir.ActivationFunctionType.Copy

    nc = bacc.Bacc("TRN2", target_bir_lowering=False, debug=False,
                   num_devices=NCORES)

    xT = nc.dram_tensor("xT", [D, BSH], fp32, kind="ExternalInput").ap()
    wencT = nc.dram_tensor("wencT", [D, H], fp32, kind="ExternalInput").ap()
    benc = nc.dram_tensor("benc", [1, H], fp32, kind="ExternalInput").ap()
    wdecT = nc.dram_tensor("wdecT", [H, O], bf16, kind="ExternalInput").ap()
    bdec = nc.dram_tensor("bdec", [1, O], fp32, kind="ExternalInput").ap()
    enc_out = nc.dram_tensor("enc_sparse", [BSH, H], fp32,
                             kind="ExternalOutput").ap()
    dec_out = nc.dram_tensor("dec", [BSH, O], fp32,
                             kind="ExternalOutput").ap()

    with tile.TileContext(nc) as tc, ExitStack() as ctx:
        const = ctx.enter_context(tc.tile_pool(name="const", bufs=1))
        dram = ctx.enter_context(tc.tile_pool(name="dram", bufs=1,
                                              space="DRAM"))

        ident = const.tile([128, 128], fp32)
        make_identity(nc, ident)
        ones1 = const.tile([1, 128], fp32)
        nc.vector.memset(ones1, 1.0)
        bdec_sb = const.tile([1, O], fp32)
        nc.sync.dma_start(out=bdec_sb, in_=bdec)

        enc_dram = dram.tile([BSH, H], fp32)
        encT_dram = dram.tile([H, BSH], bf16)

        # ---------------- Phase A: encoder ----------------
        with tc.tile_pool(name="xTp", bufs=1) as xpool, \
             tc.tile_pool(name="wenc", bufs=2) as wpool, \
             tc.tile_pool(name="encch", bufs=3) as cpool, \
             tc.tile_pool(name="psA", bufs=4, space="PSUM") as psA:
            xT_sb = xpool.tile([128, NDC, BSH], fp32)
            nc.sync.dma_start(out=xT_sb,
                              in_=xT.rearrange("(j p) b -> p j b", p=128))
            for c in range(NHC):
                w_sb = wpool.tile([128, NDC, HCH], fp32)
                nc.sync.dma_start(
                    out=w_sb,
                    in_=wencT[:, c * HCH:(c + 1) * HCH].rearrange(
                        "(j p) h -> p j h", p=128))
                benc_sb = wpool.tile([1, HCH], fp32, tag="bencch",
                                     name=f"bencch{c}")
                nc.sync.dma_start(out=benc_sb,
                                  in_=benc[:, c * HCH:(c + 1) * HCH])
                for t in range(NBT):
                    ps = psA.tile([128, HCH], fp32)
                    for d in range(NDC):
                        nc.tensor.matmul(
                            ps,
                            lhsT=xT_sb[:, d, t * 128:(t + 1) * 128],
                            rhs=w_sb[:, d, :],
                            start=(d == 0), stop=False)
                    nc.tensor.matmul(
                        ps, lhsT=ones1, rhs=benc_sb,
                        start=False, stop=True)
                    ch = cpool.tile([128, HCH], fp32)
                    nc.scalar.activation(ch, ps, RELU)
                    nc.sync.dma_start(
                        out=enc_dram[t * 128:(t + 1) * 128,
                                     c * HCH:(c + 1) * HCH],
                        in_=ch)

        # ---------------- Phase B: top-k + transpose ----------------
        with tc.tile_pool(name="encrow", bufs=1) as epool, \
             tc.tile_pool(name="work", bufs=1) as wkpool, \
             tc.tile_pool(name="encT", bufs=1) as etpool, \
             tc.tile_pool(name="m8", bufs=8) as mpool, \
             tc.tile_pool(name="psT", bufs=4, space="PSUM") as psT:
            for t in range(NBT):
                enc = epool.tile([128, H], fp32)
                nc.sync.dma_start(out=enc,
                                  in_=enc_dram[t * 128:(t + 1) * 128, :])
                work = wkpool.tile([128, H], fp32)
                src = enc
                for r in range(K // 8):
                    m = mpool.tile([128, 8], fp32)
                    nc.vector.max(out=m, in_=src)
                    nc.vector.match_replace(out=work, in_to_replace=m,
                                            in_values=src, imm_value=0.0)
                    src = work
                # enc := enc - work  == values at top-32 positions, else 0
                nc.vector.tensor_sub(out=enc, in0=enc, in1=work)
                nc.sync.dma_start(out=enc_out[t * 128:(t + 1) * 128, :],
                                  in_=enc)
                encT_sb = etpool.tile([128, 128, 128], bf16)
                for j in range(128):
                    pst = psT.tile([128, 128], fp32)
                    nc.tensor.transpose(pst, enc[:, j * 128:(j + 1) * 128],
                                        ident)
                    nc.scalar.activation(encT_sb[:, j, :], pst, COPY)
                nc.sync.dma_start(
                    out=encT_dram.rearrange("(j p) b -> p j b",
                                            p=128)[:, :,
                                                   t * 128:(t + 1) * 128],
                    in_=encT_sb)

        # ---------------- Phase C: decoder ----------------
        with tc.tile_pool(name="wdec", bufs=3) as wdpool, \
             tc.tile_pool(name="encTc", bufs=3) as ecpool, \
             tc.tile_pool(name="decout", bufs=3) as dpool, \
             tc.tile_pool(name="psC", bufs=8, space="PSUM") as psC:
            for oh in range(2):
                pss = [psC.tile([128, 512], fp32, tag="psdec",
                                name=f"psdec{oh}_{i}")
                       for i in range(NBT)]
                for c in range(H // 128):
                    wd = wdpool.tile([128, 512], bf16)
                    nc.sync.dma_start(
                        out=wd,
                        in_=wdecT[c * 128:(c + 1) * 128,
                                  oh * 512:(oh + 1) * 512])
                    et = ecpool.tile([128, BSH], bf16)
                    nc.sync.dma_start(
                        out=et, in_=encT_dram[c * 128:(c + 1) * 128, :])
                    for t in range(NBT):
                        nc.tensor.matmul(
                            pss[t], lhsT=et[:, t * 128:(t + 1) * 128],
                            rhs=wd, start=(c == 0), stop=False)
                for t in range(NBT):
                    nc.tensor.matmul(
                        pss[t], lhsT=ones1,
                        rhs=bdec_sb[:, oh * 512:(oh + 1) * 512],
                        start=False, stop=True)
                    do = dpool.tile([128, 512], fp32)
                    nc.scalar.activation(do, pss[t], COPY)
                    nc.sync.dma_start(
                        out=dec_out[t * 128:(t + 1) * 128,
                                    oh * 512:(oh + 1) * 512],
                        in_=do)

    nc.compile()
    return nc


def kernel(x, W_enc, b_enc, W_dec, b_dec, topk):
    assert int(topk) == K
    from concourse.bass_utils import run_bass_kernel_spmd

    x = np.asarray(x, dtype=np.float32)
    W_enc = np.asarray(W_enc, dtype=np.float32)
    b_enc = np.asarray(b_enc, dtype=np.float32)
    W_dec = np.asarray(W_dec, dtype=np.float32)
    b_dec = np.asarray(b_dec, dtype=np.float32)

    if "nc" not in _cache:
        _cache["nc"] = _build()
    nc = _cache["nc"]

    xT = np.ascontiguousarray(x.T)  # [D, B]
    wencT = np.ascontiguousarray(W_enc.T)  # [D, H]
    wdecT = np.ascontiguousarray(W_dec.T).astype(ml_dtypes.bfloat16)  # [H, O]
    benc = np.ascontiguousarray(b_enc.reshape(1, H))
    bdec = np.ascontiguousarray(b_dec.reshape(1, O))

    in_maps = []
    for c in range(NCORES):
        in_maps.append({
            "xT": np.ascontiguousarray(xT[:, c * BSH:(c + 1) * BSH]),
            "wencT": wencT,
            "benc": benc,
            "wdecT": wdecT,
            "bdec": bdec,
        })

    res = run_bass_kernel_spmd(nc, in_maps, core_ids=list(range(NCORES)),
                               trace=TRACE)
    LAST_RESULTS["exec_time_ns"] = res.exec_time_ns
    LAST_RESULTS["profile_json"] = res.profile_json

    enc_sparse = np.concatenate([res.results[c]["enc_sparse"]
                                 for c in range(NCORES)], axis=0)
    dec = np.concatenate([res.results[c]["dec"]
                          for c in range(NCORES)], axis=0)
    return enc_sparse.astype(np.float32), dec.astype(np.float32)


# revision 9
# speedup vs baseline: 1.0410x; 1.0410x over previous
"""Trainium2 Bass kernel for sparse autoencoder (topk masking).

  encoder:  pre = x @ W_enc.T + b_enc ; enc = relu(pre)
  topk:     per-row top-32 of enc kept, rest zeroed  -> encoded_sparse
  decoder:  dec = encoded_sparse @ W_dec.T + b_dec
  returns (encoded_sparse, dec)

Sharding: pure data-parallel over the batch dim across 8 NeuronCores
(1024 rows per core).  No collectives needed.

v2: bf16 hi/lo split encoder (3 bf16 matmuls ~= fp32 accuracy at 3/4
the PE cost), batch split into 2 groups of 512 rows so group 0's
VectorE top-k + decode overlap group 1's encoder matmuls.
"""

import sys

sys.path.insert(0, "/opt/trn_rl_repo")

import numpy as np
import ml_dtypes

B, D, H, O, K = 8192, 1024, 16384, 1024, 32
NCORES = 8
BSH = B // NCORES  # 1024 rows per core
NG = 2             # batch groups per core
GB = BSH // NG     # 512 rows per group
NBT = GB // 128    # 4 row-tiles per group
HCH = 512          # encoder h-chunk
NHC = H // HCH     # 32
NDC = D // 128     # 8 contraction chunks

TRACE = False
LAST_RESULTS = {}

_cache = {}


def _build():
    import concourse.bass as bass  # noqa: F401
    import concourse.mybir as mybir
    import concourse.tile as tile
    from concourse import bacc
    from concourse.masks import make_identity
    from contextlib import ExitStack

    fp32 = mybir.dt.float32
    bf16 = mybir.dt.bfloat16
    RELU = mybir.ActivationFunctionType.Relu
    COPY = mybir.ActivationFunctionType.Copy

    nc = bacc.Bacc("TRN2", target_bir_lowering=False, debug=False,
                   num_devices=NCORES)

    xhi = nc.dram_tensor("xhi", [D, BSH], bf16, kind="ExternalInput").ap()
    xlo = nc.dram_tensor("xlo", [D, BSH], bf16, kind="ExternalInput").ap()
    whi = nc.dram_tensor("whi", [D, H], bf16, kind="ExternalInput").ap()
    wlo = nc.dram_tensor("wlo", [D, H], bf16, kind="ExternalInput").ap()
    bhi = nc.dram_tensor("bhi", [1, H], bf16, kind="ExternalInput").ap()
    blo = nc.dram_tensor("blo", [1, H], bf16, kind="ExternalInput").ap()
    wdecT = nc.dram_tensor("wdecT", [H, O], bf16, kind="ExternalInput").ap()
    bdec = nc.dram_tensor("bdec", [1, O], fp32, kind="ExternalInput").ap()
    enc_out = nc.dram_tensor("enc_sparse", [BSH, H], fp32,
                             kind="ExternalOutput").ap()
    dec_out = nc.dram_tensor("dec", [BSH, O], fp32,
                             kind="ExternalOutput").ap()

    with tile.TileContext(nc) as tc, ExitStack() as ctx:
        const = ctx.enter_context(tc.tile_pool(name="const", bufs=1))
        dram = ctx.enter_context(tc.tile_pool(name="dram", bufs=1,
                                              space="DRAM"))
        xpool = ctx.enter_context(tc.tile_pool(name="xTp", bufs=1))
        wpool = ctx.enter_context(tc.tile_pool(name="wenc", bufs=2))
        cpool = ctx.enter_context(tc.tile_pool(name="encch", bufs=2))
        epool = ctx.enter_context(tc.tile_pool(name="encrow", bufs=1))
        wkpool = ctx.enter_context(tc.tile_pool(name="work", bufs=1))
        etpool = ctx.enter_context(tc.tile_pool(name="encT", bufs=1))
        mpool = ctx.enter_context(tc.tile_pool(name="m8", bufs=8))
        wdpool = ctx.enter_context(tc.tile_pool(name="wdec", bufs=3))
        ecpool = ctx.enter_context(tc.tile_pool(name="encTc", bufs=3))
        dpool = ctx.enter_context(tc.tile_pool(name="decout", bufs=1))
        psA = ctx.enter_context(tc.tile_pool(name="psA", bufs=3,
                                             space="PSUM"))
        psC = ctx.enter_context(tc.tile_pool(name="psC", bufs=4,
                                             space="PSUM"))

        ident = const.tile([128, 128], fp32)
        make_identity(nc, ident)
        ones_bf = const.tile([1, 128], bf16)
        nc.vector.memset(ones_bf, 1.0)
        ones_f32 = const.tile([1, 128], fp32)
        nc.vector.memset(ones_f32, 1.0)
        bdec_sb = const.tile([1, O], fp32)
        nc.sync.dma_start(out=bdec_sb, in_=bdec)

        enc_dram = [dram.tile([GB, H], fp32, name=f"enc_dram{g}")
                    for g in range(NG)]
        encT_dram = [dram.tile([H, GB], bf16, name=f"encT_dram{g}")
                     for g in range(NG)]

        for g in range(NG):
            gc = slice(g * GB, (g + 1) * GB)  # this group's batch columns
            # ---------------- Phase A(g): encoder ----------------
            xhi_sb = xpool.tile([128, NDC, GB], bf16, tag="xhi",
                                name=f"xhi{g}")
            nc.sync.dma_start(
                out=xhi_sb,
                in_=xhi[:, gc].rearrange("(j p) b -> p j b", p=128))
            xlo_sb = xpool.tile([128, NDC, GB], bf16, tag="xlo",
                                name=f"xlo{g}")
            nc.sync.dma_start(
                out=xlo_sb,
                in_=xlo[:, gc].rearrange("(j p) b -> p j b", p=128))
            for c in range(NHC):
                hs = slice(c * HCH, (c + 1) * HCH)
                whi_sb = wpool.tile([128, NDC, HCH], bf16, tag="whi",
                                    name=f"whi{g}_{c}")
                nc.sync.dma_start(
                    out=whi_sb,
                    in_=whi[:, hs].rearrange("(j p) h -> p j h", p=128))
                wlo_sb = wpool.tile([128, NDC, HCH], bf16, tag="wlo",
                                    name=f"wlo{g}_{c}")
                nc.sync.dma_start(
                    out=wlo_sb,
                    in_=wlo[:, hs].rearrange("(j p) h -> p j h", p=128))
                bhi_sb = wpool.tile([1, HCH], bf16, tag="bhi",
                                    name=f"bhi{g}_{c}")
                nc.sync.dma_start(out=bhi_sb, in_=bhi[:, hs])
                blo_sb = wpool.tile([1, HCH], bf16, tag="blo",
                                    name=f"blo{g}_{c}")
                nc.sync.dma_start(out=blo_sb, in_=blo[:, hs])
                for t in range(NBT):
                    ts_ = slice(t * 128, (t + 1) * 128)
                    ps = psA.tile([128, HCH], fp32, tag="psa",
                                  name=f"psa{g}_{c}_{t}")
                    for d in range(NDC):
                        nc.tensor.matmul(ps, lhsT=xhi_sb[:, d, ts_],
                                         rhs=whi_sb[:, d, :],
                                         start=(d == 0), stop=False)
                        nc.tensor.matmul(ps, lhsT=xhi_sb[:, d, ts_],
                                         rhs=wlo_sb[:, d, :],
                                         start=False, stop=False)
                        nc.tensor.matmul(ps, lhsT=xlo_sb[:, d, ts_],
                                         rhs=whi_sb[:, d, :],
                                         start=False, stop=False)
                    nc.tensor.matmul(ps, lhsT=ones_bf, rhs=bhi_sb,
                                     start=False, stop=False)
                    nc.tensor.matmul(ps, lhsT=ones_bf, rhs=blo_sb,
                                     start=False, stop=True)
                    ch = cpool.tile([128, HCH], fp32, tag="encch",
                                    name=f"ch{g}_{c}_{t}")
                    nc.scalar.activation(ch, ps, RELU)
                    nc.sync.dma_start(out=enc_dram[g][ts_, hs], in_=ch)

            # ---------------- Phase B(g): top-k + transpose ----------
            for t in range(NBT):
                ts_ = slice(t * 128, (t + 1) * 128)
                enc = epool.tile([128, H], fp32, tag="enc",
                                 name=f"enc{g}_{t}")
                nc.sync.dma_start(out=enc, in_=enc_dram[g][ts_, :])
                work = wkpool.tile([128, H], fp32, tag="work",
                                   name=f"work{g}_{t}")
                src = enc
                for r in range(K // 8):
                    m = mpool.tile([128, 8], fp32, tag="m8",
                                   name=f"m{g}_{t}_{r}")
                    nc.vector.max(out=m, in_=src)
                    nc.vector.match_replace(out=work, in_to_replace=m,
                                            in_values=src, imm_value=0.0)
                    src = work
                nc.vector.tensor_sub(out=enc, in0=enc, in1=work)
                nc.sync.dma_start(
                    out=enc_out[g * GB + t * 128:g * GB + (t + 1) * 128, :],
                    in_=enc)
                # transpose masked tile -> bf16 encT (quarters)
                for q in range(4):
                    encT_sb = etpool.tile([128, 32, 128], bf16, tag="encT",
                                          name=f"encT{g}_{t}_{q}")
                    for j in range(32):
                        jj = q * 32 + j
                        pst = psA.tile([128, 128], fp32, tag="psa",
                                       name=f"pst{g}_{t}_{q}_{j}")
                        nc.tensor.transpose(
                            pst, enc[:, jj * 128:(jj + 1) * 128], ident)
                        nc.scalar.activation(encT_sb[:, j, :], pst, COPY)
                    nc.sync.dma_start(
                        out=encT_dram[g].rearrange(
                            "(j p) b -> p j b",
                            p=128)[:, q * 32:(q + 1) * 32, ts_],
                        in_=encT_sb)

            # ---------------- Phase C(g): decoder ----------------
            for oh in range(2):
                os_ = slice(oh * 512, (oh + 1) * 512)
                pss = [psC.tile([128, 512], fp32, tag="psdec",
                                name=f"psdec{g}_{oh}_{i}")
                       for i in range(NBT)]
                for c in range(H // 128):
                    cs = slice(c * 128, (c + 1) * 128)
                    wd = wdpool.tile([128, 512], bf16, tag="wd",
                                     name=f"wd{g}_{oh}_{c}")
                    nc.sync.dma_start(out=wd, in_=wdecT[cs, os_])
                    et = ecpool.tile([128, GB], bf16, tag="et",
                                     name=f"et{g}_{oh}_{c}")
                    nc.sync.dma_start(out=et, in_=encT_dram[g][cs, :])
                    for t in range(NBT):
                        nc.tensor.matmul(
                            pss[t], lhsT=et[:, t * 128:(t + 1) * 128],
                            rhs=wd, start=(c == 0), stop=False)
                for t in range(NBT):
                    nc.tensor.matmul(pss[t], lhsT=ones_f32,
                                     rhs=bdec_sb[:, os_],
                                     start=False, stop=True)
                    do = dpool.tile([128, 512], fp32, tag="do",
                                    name=f"do{g}_{oh}_{t}")
                    nc.scalar.activation(do, pss[t], COPY)
                    nc.sync.dma_start(
                        out=dec_out[g * GB + t * 128:g * GB + (t + 1) * 128,
                                    os_],
                        in_=do)

    nc.compile()
    return nc


def _split_bf16(a):
    hi = a.astype(ml_dtypes.bfloat16)
    lo = (a - hi.astype(np.float32)).astype(ml_dtypes.bfloat16)
    return hi, lo


def kernel(x, W_enc, b_enc, W_dec, b_dec, topk):
    assert int(topk) == K
    from concourse.bass_utils import run_bass_kernel_spmd

    x = np.asarray(x, dtype=np.float32)
    W_enc = np.asarray(W_enc, dtype=np.float32)
    b_enc = np.asarray(b_enc, dtype=np.float32)
    W_dec = np.asarray(W_dec, dtype=np.float32)
    b_dec = np.asarray(b_dec, dtype=np.float32)

    if "nc" not in _cache:
        _cache["nc"] = _build()
    nc = _cache["nc"]

    xT = np.ascontiguousarray(x.T)  # [D, B]
    xT_hi, xT_lo = _split_bf16(xT)
    wencT = np.ascontiguousarray(W_enc.T)  # [D, H]
    w_hi, w_lo = _split_bf16(wencT)
    b_hi, b_lo = _split_bf16(b_enc.reshape(1, H))
    wdecT = np.ascontiguousarray(W_dec.T).astype(ml_dtypes.bfloat16)
    bdec = np.ascontiguousarray(b_dec.reshape(1, O))

    in_maps = []
    for c in range(NCORES):
        cs = slice(c * BSH, (c + 1) * BSH)
        in_maps.append({
            "xhi": np.ascontiguousarray(xT_hi[:, cs]),
            "xlo": np.ascontiguousarray(xT_lo[:, cs]),
            "whi": w_hi,
            "wlo": w_lo,
            "bhi": b_hi,
            "blo": b_lo,
            "wdecT": wdecT,
            "bdec": bdec,
        })

    res = run_bass_kernel_spmd(nc, in_maps, core_ids=list(range(NCORES)),
                               trace=TRACE)
    LAST_RESULTS["exec_time_ns"] = res.exec_time_ns
    LAST_RESULTS["profile_json"] = res.profile_json

    enc_sparse = np.concatenate([res.results[c]["enc_sparse"]
                                 for c in range(NCORES)], axis=0)
    dec = np.concatenate([res.results[c]["dec"]
                          for c in range(NCORES)], axis=0)
    return enc_sparse.astype(np.float32), dec.astype(np.float32)


# revision 10
# speedup vs baseline: 1.1921x; 1.1452x over previous
"""Trainium2 Bass kernel for sparse autoencoder (topk masking).

  encoder:  pre = x @ W_enc.T + b_enc ; enc = relu(pre)
  topk:     per-row top-32 of enc kept, rest zeroed  -> encoded_sparse
  decoder:  dec = encoded_sparse @ W_dec.T + b_dec
  returns (encoded_sparse, dec)

Sharding: pure data-parallel over the batch dim across 8 NeuronCores
(1024 rows per core).  No collectives needed.

v2: bf16 hi/lo split encoder (3 bf16 matmuls ~= fp32 accuracy at 3/4
the PE cost), batch split into 2 groups of 512 rows so group 0's
VectorE top-k + decode overlap group 1's encoder matmuls.
"""

import sys

sys.path.insert(0, "/opt/trn_rl_repo")

import numpy as np
import ml_dtypes

B, D, H, O, K = 8192, 1024, 16384, 1024, 32
NCORES = 8
BSH = B // NCORES  # 1024 rows per core
NG = 2             # batch groups per core
GB = BSH // NG     # 512 rows per group
NBT = GB // 128    # 4 row-tiles per group
HCH = 512          # encoder h-chunk
NHC = H // HCH     # 32
NDC = D // 128     # 8 contraction chunks

TRACE = False
LAST_RESULTS = {}

_cache = {}


def _build():
    import concourse.bass as bass  # noqa: F401
    import concourse.mybir as mybir
    import concourse.tile as tile
    from concourse import bacc
    from concourse.masks import make_identity
    from contextlib import ExitStack

    fp32 = mybir.dt.float32
    bf16 = mybir.dt.bfloat16
    RELU = mybir.ActivationFunctionType.Relu
    COPY = mybir.ActivationFunctionType.Copy

    nc = bacc.Bacc("TRN2", target_bir_lowering=False, debug=False,
                   num_devices=NCORES)

    xhi = nc.dram_tensor("xhi", [D, BSH], bf16, kind="ExternalInput").ap()
    xlo = nc.dram_tensor("xlo", [D, BSH], bf16, kind="ExternalInput").ap()
    whi = nc.dram_tensor("whi", [D, H], bf16, kind="ExternalInput").ap()
    wlo = nc.dram_tensor("wlo", [D, H], bf16, kind="ExternalInput").ap()
    bhi = nc.dram_tensor("bhi", [1, H], bf16, kind="ExternalInput").ap()
    blo = nc.dram_tensor("blo", [1, H], bf16, kind="ExternalInput").ap()
    wdecT = nc.dram_tensor("wdecT", [H, O], bf16, kind="ExternalInput").ap()
    bdec = nc.dram_tensor("bdec", [1, O], fp32, kind="ExternalInput").ap()
    enc_out = nc.dram_tensor("enc_sparse", [BSH, H], fp32,
                             kind="ExternalOutput").ap()
    dec_out = nc.dram_tensor("dec", [BSH, O], fp32,
                             kind="ExternalOutput").ap()

    with tile.TileContext(nc) as tc, ExitStack() as ctx:
        const = ctx.enter_context(tc.tile_pool(name="const", bufs=1))
        dram = ctx.enter_context(tc.tile_pool(name="dram", bufs=1,
                                              space="DRAM"))
        xpool = ctx.enter_context(tc.tile_pool(name="xTp", bufs=1))
        wpool = ctx.enter_context(tc.tile_pool(name="wenc", bufs=2))
        cpool = ctx.enter_context(tc.tile_pool(name="encch", bufs=2))
        epool = ctx.enter_context(tc.tile_pool(name="encrow", bufs=1))
        wkpool = ctx.enter_context(tc.tile_pool(name="work", bufs=1))
        etpool = ctx.enter_context(tc.tile_pool(name="encT", bufs=1))
        mpool = ctx.enter_context(tc.tile_pool(name="m8", bufs=8))
        wdpool = ctx.enter_context(tc.tile_pool(name="wdec", bufs=3))
        ecpool = ctx.enter_context(tc.tile_pool(name="encTc", bufs=3))
        dpool = ctx.enter_context(tc.tile_pool(name="decout", bufs=1))
        psA = ctx.enter_context(tc.tile_pool(name="psA", bufs=2,
                                             space="PSUM"))
        psT = ctx.enter_context(tc.tile_pool(name="psT", bufs=2,
                                             space="PSUM"))
        psC = ctx.enter_context(tc.tile_pool(name="psC", bufs=4,
                                             space="PSUM"))

        ident = const.tile([128, 128], fp32)
        make_identity(nc, ident)
        ones_bf = const.tile([1, 128], bf16)
        nc.vector.memset(ones_bf, 1.0)
        ones_f32 = const.tile([1, 128], fp32)
        nc.vector.memset(ones_f32, 1.0)
        bdec_sb = const.tile([1, O], fp32)
        nc.sync.dma_start(out=bdec_sb, in_=bdec)

        enc_dram = [dram.tile([GB, H], fp32, name=f"enc_dram{g}")
                    for g in range(NG)]
        encT_dram = [dram.tile([H, GB], bf16, name=f"encT_dram{g}")
                     for g in range(NG)]

        for g in range(NG):
            gc = slice(g * GB, (g + 1) * GB)  # this group's batch columns
            # ---------------- Phase A(g): encoder ----------------
            xhi_sb = xpool.tile([128, NDC, GB], bf16, tag="xhi",
                                name=f"xhi{g}")
            nc.sync.dma_start(
                out=xhi_sb,
                in_=xhi[:, gc].rearrange("(j p) b -> p j b", p=128))
            xlo_sb = xpool.tile([128, NDC, GB], bf16, tag="xlo",
                                name=f"xlo{g}")
            nc.sync.dma_start(
                out=xlo_sb,
                in_=xlo[:, gc].rearrange("(j p) b -> p j b", p=128))
            for c in range(NHC):
                hs = slice(c * HCH, (c + 1) * HCH)
                whi_sb = wpool.tile([128, NDC, HCH], bf16, tag="whi",
                                    name=f"whi{g}_{c}")
                nc.sync.dma_start(
                    out=whi_sb,
                    in_=whi[:, hs].rearrange("(j p) h -> p j h", p=128))
                wlo_sb = wpool.tile([128, NDC, HCH], bf16, tag="wlo",
                                    name=f"wlo{g}_{c}")
                nc.sync.dma_start(
                    out=wlo_sb,
                    in_=wlo[:, hs].rearrange("(j p) h -> p j h", p=128))
                bhi_sb = wpool.tile([1, HCH], bf16, tag="bhi",
                                    name=f"bhi{g}_{c}")
                nc.sync.dma_start(out=bhi_sb, in_=bhi[:, hs])
                blo_sb = wpool.tile([1, HCH], bf16, tag="blo",
                                    name=f"blo{g}_{c}")
                nc.sync.dma_start(out=blo_sb, in_=blo[:, hs])
                for t in range(NBT):
                    ts_ = slice(t * 128, (t + 1) * 128)
                    ps = psA.tile([128, HCH], fp32, tag="psa",
                                  name=f"psa{g}_{c}_{t}")
                    for d in range(NDC):
                        nc.tensor.matmul(ps, lhsT=xhi_sb[:, d, ts_],
                                         rhs=whi_sb[:, d, :],
                                         start=(d == 0), stop=False)
                        nc.tensor.matmul(ps, lhsT=xhi_sb[:, d, ts_],
                                         rhs=wlo_sb[:, d, :],
                                         start=False, stop=False)
                        nc.tensor.matmul(ps, lhsT=xlo_sb[:, d, ts_],
                                         rhs=whi_sb[:, d, :],
                                         start=False, stop=False)
                    nc.tensor.matmul(ps, lhsT=ones_bf, rhs=bhi_sb,
                                     start=False, stop=False)
                    nc.tensor.matmul(ps, lhsT=ones_bf, rhs=blo_sb,
                                     start=False, stop=True)
                    ch = cpool.tile([128, HCH], fp32, tag="encch",
                                    name=f"ch{g}_{c}_{t}")
                    nc.scalar.activation(ch, ps, RELU)
                    nc.sync.dma_start(out=enc_dram[g][ts_, hs], in_=ch)

            # ---------------- Phase B(g): top-k + transpose ----------
            for t in range(NBT):
                ts_ = slice(t * 128, (t + 1) * 128)
                enc = epool.tile([128, H], fp32, tag="enc",
                                 name=f"enc{g}_{t}")
                nc.sync.dma_start(out=enc, in_=enc_dram[g][ts_, :])
                work = wkpool.tile([128, H], fp32, tag="work",
                                   name=f"work{g}_{t}")
                src = enc
                for r in range(K // 8):
                    m = mpool.tile([128, 8], fp32, tag="m8",
                                   name=f"m{g}_{t}_{r}")
                    nc.vector.max(out=m, in_=src)
                    nc.vector.match_replace(out=work, in_to_replace=m,
                                            in_values=src, imm_value=0.0)
                    src = work
                nc.gpsimd.tensor_sub(out=enc, in0=enc, in1=work)
                nc.sync.dma_start(
                    out=enc_out[g * GB + t * 128:g * GB + (t + 1) * 128, :],
                    in_=enc)
                # transpose masked tile -> bf16 encT (quarters)
                for q in range(4):
                    encT_sb = etpool.tile([128, 32, 128], bf16, tag="encT",
                                          name=f"encT{g}_{t}_{q}")
                    for j in range(32):
                        jj = q * 32 + j
                        pst = psT.tile([128, 128], fp32, tag="pst",
                                       name=f"pst{g}_{t}_{q}_{j}")
                        nc.tensor.transpose(
                            pst, enc[:, jj * 128:(jj + 1) * 128], ident)
                        nc.scalar.activation(encT_sb[:, j, :], pst, COPY)
                    nc.sync.dma_start(
                        out=encT_dram[g].rearrange(
                            "(j p) b -> p j b",
                            p=128)[:, q * 32:(q + 1) * 32, ts_],
                        in_=encT_sb)

            # ---------------- Phase C(g): decoder ----------------
            for oh in range(2):
                os_ = slice(oh * 512, (oh + 1) * 512)
                pss = [psC.tile([128, 512], fp32, tag="psdec",
                                name=f"psdec{g}_{oh}_{i}")
                       for i in range(NBT)]
                for c in range(H // 128):
                    cs = slice(c * 128, (c + 1) * 128)
                    wd = wdpool.tile([128, 512], bf16, tag="wd",
                                     name=f"wd{g}_{oh}_{c}")
                    nc.sync.dma_start(out=wd, in_=wdecT[cs, os_])
                    et = ecpool.tile([128, GB], bf16, tag="et",
                                     name=f"et{g}_{oh}_{c}")
                    nc.sync.dma_start(out=et, in_=encT_dram[g][cs, :])
                    for t in range(NBT):
                        nc.tensor.matmul(
                            pss[t], lhsT=et[:, t * 128:(t + 1) * 128],
                            rhs=wd, start=(c == 0), stop=False)
                for t in range(NBT):
                    nc.tensor.matmul(pss[t], lhsT=ones_f32,
                                     rhs=bdec_sb[:, os_],
                                     start=False, stop=True)
                    do = dpool.tile([128, 512], fp32, tag="do",
                                    name=f"do{g}_{oh}_{t}")
                    nc.scalar.activation(do, pss[t], COPY)
                    nc.sync.dma_start(
                        out=dec_out[g * GB + t * 128:g * GB + (t + 1) * 128,
                                    os_],
                        in_=do)

    nc.compile()
    return nc


def _split_bf16(a):
    hi = a.astype(ml_dtypes.bfloat16)
    lo = (a - hi.astype(np.float32)).astype(ml_dtypes.bfloat16)
    return hi, lo


def kernel(x, W_enc, b_enc, W_dec, b_dec, topk):
    assert int(topk) == K
    from concourse.bass_utils import run_bass_kernel_spmd

    x = np.asarray(x, dtype=np.float32)
    W_enc = np.asarray(W_enc, dtype=np.float32)
    b_enc = np.asarray(b_enc, dtype=np.float32)
    W_dec = np.asarray(W_dec, dtype=np.float32)
    b_dec = np.asarray(b_dec, dtype=np.float32)

    if "nc" not in _cache:
        _cache["nc"] = _build()
    nc = _cache["nc"]

    xT = np.ascontiguousarray(x.T)  # [D, B]
    xT_hi, xT_lo = _split_bf16(xT)
    wencT = np.ascontiguousarray(W_enc.T)  # [D, H]
    w_hi, w_lo = _split_bf16(wencT)
    b_hi, b_lo = _split_bf16(b_enc.reshape(1, H))
    wdecT = np.ascontiguousarray(W_dec.T).astype(ml_dtypes.bfloat16)
    bdec = np.ascontiguousarray(b_dec.reshape(1, O))

    in_maps = []
    for c in range(NCORES):
        cs = slice(c * BSH, (c + 1) * BSH)
        in_maps.append({
            "xhi": np.ascontiguousarray(xT_hi[:, cs]),
            "xlo": np.ascontiguousarray(xT_lo[:, cs]),
            "whi": w_hi,
            "wlo": w_lo,
            "bhi": b_hi,
            "blo": b_lo,
            "wdecT": wdecT,
            "bdec": bdec,
        })

    res = run_bass_kernel_spmd(nc, in_maps, core_ids=list(range(NCORES)),
                               trace=TRACE)
    LAST_RESULTS["exec_time_ns"] = res.exec_time_ns
    LAST_RESULTS["profile_json"] = res.profile_json

    enc_sparse = np.concatenate([res.results[c]["enc_sparse"]
                                 for c in range(NCORES)], axis=0)
    dec = np.concatenate([res.results[c]["dec"]
                          for c in range(NCORES)], axis=0)
    return enc_sparse.astype(np.float32), dec.astype(np.float32)


# revision 11
# speedup vs baseline: 1.2811x; 1.0746x over previous
"""Trainium2 Bass kernel for sparse autoencoder (topk masking).

  encoder:  pre = x @ W_enc.T + b_enc ; enc = relu(pre)
  topk:     per-row top-32 of enc kept, rest zeroed  -> encoded_sparse
  decoder:  dec = encoded_sparse @ W_dec.T + b_dec
  returns (encoded_sparse, dec)

Sharding: pure data-parallel over the batch dim across 8 NeuronCores
(1024 rows per core).  No collectives needed.

v2: bf16 hi/lo split encoder (3 bf16 matmuls ~= fp32 accuracy at 3/4
the PE cost), batch split into 2 groups of 512 rows so group 0's
VectorE top-k + decode overlap group 1's encoder matmuls.
"""

import sys

sys.path.insert(0, "/opt/trn_rl_repo")

import numpy as np
import ml_dtypes

B, D, H, O, K = 8192, 1024, 16384, 1024, 32
NCORES = 8
BSH = B // NCORES  # 1024 rows per core
NG = 2             # batch groups per core
GB = BSH // NG     # 512 rows per group
NBT = GB // 128    # 4 row-tiles per group
HCH = 512          # encoder h-chunk
NHC = H // HCH     # 32
NDC = D // 128     # 8 contraction chunks

TRACE = False
LAST_RESULTS = {}

_cache = {}


def _build():
    import concourse.bass as bass  # noqa: F401
    import concourse.mybir as mybir
    import concourse.tile as tile
    from concourse import bacc
    from concourse.masks import make_identity
    from contextlib import ExitStack

    fp32 = mybir.dt.float32
    bf16 = mybir.dt.bfloat16
    RELU = mybir.ActivationFunctionType.Relu
    COPY = mybir.ActivationFunctionType.Copy

    nc = bacc.Bacc("TRN2", target_bir_lowering=False, debug=False,
                   num_devices=NCORES)

    xhi = nc.dram_tensor("xhi", [D, BSH], bf16, kind="ExternalInput").ap()
    xlo = nc.dram_tensor("xlo", [D, BSH], bf16, kind="ExternalInput").ap()
    whi = nc.dram_tensor("whi", [D, H], bf16, kind="ExternalInput").ap()
    wlo = nc.dram_tensor("wlo", [D, H], bf16, kind="ExternalInput").ap()
    bhi = nc.dram_tensor("bhi", [1, H], bf16, kind="ExternalInput").ap()
    blo = nc.dram_tensor("blo", [1, H], bf16, kind="ExternalInput").ap()
    wdecT = nc.dram_tensor("wdecT", [H, O], bf16, kind="ExternalInput").ap()
    bdec = nc.dram_tensor("bdec", [1, O], fp32, kind="ExternalInput").ap()
    enc_out = nc.dram_tensor("enc_sparse", [BSH, H], fp32,
                             kind="ExternalOutput").ap()
    dec_out = nc.dram_tensor("dec", [BSH, O], fp32,
                             kind="ExternalOutput").ap()

    with tile.TileContext(nc) as tc, ExitStack() as ctx:
        const = ctx.enter_context(tc.tile_pool(name="const", bufs=1))
        dram = ctx.enter_context(tc.tile_pool(name="dram", bufs=1,
                                              space="DRAM"))
        xpool = ctx.enter_context(tc.tile_pool(name="xTp", bufs=1))
        wpool = ctx.enter_context(tc.tile_pool(name="wenc", bufs=2))
        cpool = ctx.enter_context(tc.tile_pool(name="encch", bufs=2))
        pp0 = ctx.enter_context(tc.tile_pool(name="ping0", bufs=1))
        pp1 = ctx.enter_context(tc.tile_pool(name="ping1", bufs=1))
        etpool = ctx.enter_context(tc.tile_pool(name="encT", bufs=1))
        mpool = ctx.enter_context(tc.tile_pool(name="m8", bufs=8))
        wdpool = ctx.enter_context(tc.tile_pool(name="wdec", bufs=3))
        ecpool = ctx.enter_context(tc.tile_pool(name="encTc", bufs=3))
        dpool = ctx.enter_context(tc.tile_pool(name="decout", bufs=1))
        psA = ctx.enter_context(tc.tile_pool(name="psA", bufs=2,
                                             space="PSUM"))
        psT = ctx.enter_context(tc.tile_pool(name="psT", bufs=2,
                                             space="PSUM"))
        psC = ctx.enter_context(tc.tile_pool(name="psC", bufs=4,
                                             space="PSUM"))

        ident = const.tile([128, 128], fp32)
        make_identity(nc, ident)
        ones_bf = const.tile([1, 128], bf16)
        nc.vector.memset(ones_bf, 1.0)
        ones_f32 = const.tile([1, 128], fp32)
        nc.vector.memset(ones_f32, 1.0)
        bdec_sb = const.tile([1, O], fp32)
        nc.sync.dma_start(out=bdec_sb, in_=bdec)

        enc_dram = [dram.tile([GB, H], fp32, name=f"enc_dram{g}")
                    for g in range(NG)]
        encT_dram = [dram.tile([H, GB], bf16, name=f"encT_dram{g}")
                     for g in range(NG)]

        for g in range(NG):
            gc = slice(g * GB, (g + 1) * GB)  # this group's batch columns
            # ---------------- Phase A(g): encoder ----------------
            xhi_sb = xpool.tile([128, NDC, GB], bf16, tag="xhi",
                                name=f"xhi{g}")
            nc.sync.dma_start(
                out=xhi_sb,
                in_=xhi[:, gc].rearrange("(j p) b -> p j b", p=128))
            xlo_sb = xpool.tile([128, NDC, GB], bf16, tag="xlo",
                                name=f"xlo{g}")
            nc.sync.dma_start(
                out=xlo_sb,
                in_=xlo[:, gc].rearrange("(j p) b -> p j b", p=128))
            for c in range(NHC):
                hs = slice(c * HCH, (c + 1) * HCH)
                whi_sb = wpool.tile([128, NDC, HCH], bf16, tag="whi",
                                    name=f"whi{g}_{c}")
                nc.sync.dma_start(
                    out=whi_sb,
                    in_=whi[:, hs].rearrange("(j p) h -> p j h", p=128))
                wlo_sb = wpool.tile([128, NDC, HCH], bf16, tag="wlo",
                                    name=f"wlo{g}_{c}")
                nc.sync.dma_start(
                    out=wlo_sb,
                    in_=wlo[:, hs].rearrange("(j p) h -> p j h", p=128))
                bhi_sb = wpool.tile([1, HCH], bf16, tag="bhi",
                                    name=f"bhi{g}_{c}")
                nc.sync.dma_start(out=bhi_sb, in_=bhi[:, hs])
                blo_sb = wpool.tile([1, HCH], bf16, tag="blo",
                                    name=f"blo{g}_{c}")
                nc.sync.dma_start(out=blo_sb, in_=blo[:, hs])
                for t in range(NBT):
                    ts_ = slice(t * 128, (t + 1) * 128)
                    ps = psA.tile([128, HCH], fp32, tag="psa",
                                  name=f"psa{g}_{c}_{t}")
                    for d in range(NDC):
                        nc.tensor.matmul(ps, lhsT=xhi_sb[:, d, ts_],
                                         rhs=whi_sb[:, d, :],
                                         start=(d == 0), stop=False)
                        nc.tensor.matmul(ps, lhsT=xhi_sb[:, d, ts_],
                                         rhs=wlo_sb[:, d, :],
                                         start=False, stop=False)
                        nc.tensor.matmul(ps, lhsT=xlo_sb[:, d, ts_],
                                         rhs=whi_sb[:, d, :],
                                         start=False, stop=False)
                    nc.tensor.matmul(ps, lhsT=ones_bf, rhs=bhi_sb,
                                     start=False, stop=False)
                    nc.tensor.matmul(ps, lhsT=ones_bf, rhs=blo_sb,
                                     start=False, stop=True)
                    ch = cpool.tile([128, HCH], fp32, tag="encch",
                                    name=f"ch{g}_{c}_{t}")
                    nc.scalar.activation(ch, ps, RELU)
                    nc.scalar.dma_start(out=enc_dram[g][ts_, hs], in_=ch)

            # ---------------- Phase B(g): top-k + transpose ----------
            for t in range(NBT):
                ts_ = slice(t * 128, (t + 1) * 128)
                pe_, pw_ = (pp0, pp1) if (g * NBT + t) % 2 == 0 else (pp1, pp0)
                enc = pe_.tile([128, H], fp32, tag="big",
                               name=f"enc{g}_{t}")
                for q in range(4):
                    qs = slice(q * (H // 4), (q + 1) * (H // 4))
                    eng = (nc.sync, nc.gpsimd, nc.scalar, nc.sync)[q]
                    eng.dma_start(out=enc[:, qs], in_=enc_dram[g][ts_, qs])
                work = pw_.tile([128, H], fp32, tag="big",
                                name=f"work{g}_{t}")
                src = enc
                for r in range(K // 8):
                    m = mpool.tile([128, 8], fp32, tag="m8",
                                   name=f"m{g}_{t}_{r}")
                    nc.vector.max(out=m, in_=src)
                    nc.vector.match_replace(out=work, in_to_replace=m,
                                            in_values=src, imm_value=0.0)
                    src = work
                nc.gpsimd.tensor_sub(out=enc, in0=enc, in1=work)
                for q in range(4):
                    qs = slice(q * (H // 4), (q + 1) * (H // 4))
                    eng = (nc.scalar, nc.sync, nc.gpsimd, nc.scalar)[q]
                    eng.dma_start(
                        out=enc_out[g * GB + t * 128:g * GB + (t + 1) * 128,
                                    qs],
                        in_=enc[:, qs])
                # transpose masked tile -> bf16 encT (quarters)
                for q in range(4):
                    encT_sb = etpool.tile([128, 32, 128], bf16, tag="encT",
                                          name=f"encT{g}_{t}_{q}")
                    for j in range(32):
                        jj = q * 32 + j
                        pst = psT.tile([128, 128], fp32, tag="pst",
                                       name=f"pst{g}_{t}_{q}_{j}")
                        nc.tensor.transpose(
                            pst, enc[:, jj * 128:(jj + 1) * 128], ident)
                        nc.scalar.activation(encT_sb[:, j, :], pst, COPY)
                    nc.sync.dma_start(
                        out=encT_dram[g].rearrange(
                            "(j p) b -> p j b",
                            p=128)[:, q * 32:(q + 1) * 32, ts_],
                        in_=encT_sb)

            # ---------------- Phase C(g): decoder ----------------
            for oh in range(2):
                os_ = slice(oh * 512, (oh + 1) * 512)
                pss = [psC.tile([128, 512], fp32, tag="psdec",
                                name=f"psdec{g}_{oh}_{i}")
                       for i in range(NBT)]
                for c in range(H // 128):
                    cs = slice(c * 128, (c + 1) * 128)
                    wd = wdpool.tile([128, 512], bf16, tag="wd",
                                     name=f"wd{g}_{oh}_{c}")
                    nc.sync.dma_start(out=wd, in_=wdecT[cs, os_])
                    et = ecpool.tile([128, GB], bf16, tag="et",
                                     name=f"et{g}_{oh}_{c}")
                    nc.sync.dma_start(out=et, in_=encT_dram[g][cs, :])
                    for t in range(NBT):
                        nc.tensor.matmul(
                            pss[t], lhsT=et[:, t * 128:(t + 1) * 128],
                            rhs=wd, start=(c == 0), stop=False)
                for t in range(NBT):
                    nc.tensor.matmul(pss[t], lhsT=ones_f32,
                                     rhs=bdec_sb[:, os_],
                                     start=False, stop=True)
                    do = dpool.tile([128, 512], fp32, tag="do",
                                    name=f"do{g}_{oh}_{t}")
                    nc.scalar.activation(do, pss[t], COPY)
                    nc.sync.dma_start(
                        out=dec_out[g * GB + t * 128:g * GB + (t + 1) * 128,
                                    os_],
                        in_=do)

    nc.compile()
    return nc


def _split_bf16(a):
    hi = a.astype(ml_dtypes.bfloat16)
    lo = (a - hi.astype(np.float32)).astype(ml_dtypes.bfloat16)
    return hi, lo


def kernel(x, W_enc, b_enc, W_dec, b_dec, topk):
    assert int(topk) == K
    from concourse.bass_utils import run_bass_kernel_spmd

    x = np.asarray(x, dtype=np.float32)
    W_enc = np.asarray(W_enc, dtype=np.float32)
    b_enc = np.asarray(b_enc, dtype=np.float32)
    W_dec = np.asarray(W_dec, dtype=np.float32)
    b_dec = np.asarray(b_dec, dtype=np.float32)

    if "nc" not in _cache:
        _cache["nc"] = _build()
    nc = _cache["nc"]

    xT = np.ascontiguousarray(x.T)  # [D, B]
    xT_hi, xT_lo = _split_bf16(xT)
    wencT = np.ascontiguousarray(W_enc.T)  # [D, H]
    w_hi, w_lo = _split_bf16(wencT)
    b_hi, b_lo = _split_bf16(b_enc.reshape(1, H))
    wdecT = np.ascontiguousarray(W_dec.T).astype(ml_dtypes.bfloat16)
    bdec = np.ascontiguousarray(b_dec.reshape(1, O))

    in_maps = []
    for c in range(NCORES):
        cs = slice(c * BSH, (c + 1) * BSH)
        in_maps.append({
            "xhi": np.ascontiguousarray(xT_hi[:, cs]),
            "xlo": np.ascontiguousarray(xT_lo[:, cs]),
            "whi": w_hi,
            "wlo": w_lo,
            "bhi": b_hi,
            "blo": b_lo,
            "wdecT": wdecT,
            "bdec": bdec,
        })

    res = run_bass_kernel_spmd(nc, in_maps, core_ids=list(range(NCORES)),
                               trace=TRACE)
    LAST_RESULTS["exec_time_ns"] = res.exec_time_ns
    LAST_RESULTS["profile_json"] = res.profile_json

    enc_sparse = np.concatenate([res.results[c]["enc_sparse"]
                                 for c in range(NCORES)], axis=0)
    dec = np.concatenate([res.results[c]["dec"]
                          for c in range(NCORES)], axis=0)
    return enc_sparse.astype(np.float32), dec.astype(np.float32)


# revision 14
# speedup vs baseline: 1.5428x; 1.2043x over previous
"""Trainium2 Bass kernel for sparse autoencoder (topk masking).

  encoder:  pre = x @ W_enc.T + b_enc ; enc = relu(pre)
  topk:     per-row top-32 of enc kept, rest zeroed  -> encoded_sparse
  decoder:  dec = encoded_sparse @ W_dec.T + b_dec
  returns (encoded_sparse, dec)

Sharding: pure data-parallel over the batch dim across 8 NeuronCores
(1024 rows per core).  No collectives needed.

v2: bf16 hi/lo split encoder (3 bf16 matmuls ~= fp32 accuracy at 3/4
the PE cost), batch split into 2 groups of 512 rows so group 0's
VectorE top-k + decode overlap group 1's encoder matmuls.
"""

import sys

sys.path.insert(0, "/opt/trn_rl_repo")

import numpy as np
import ml_dtypes

B, D, H, O, K = 8192, 1024, 16384, 1024, 32
NCORES = 8
BSH = B // NCORES  # 1024 rows per core
NG = 2             # batch groups per core
GB = BSH // NG     # 512 rows per group
NBT = GB // 128    # 4 row-tiles per group
HCH = 512          # encoder h-chunk
NHC = H // HCH     # 32
NDC = D // 128     # 8 contraction chunks
SEG = 256          # top-k candidate segment width
NSEG = H // SEG    # 64 segments -> 512 candidates/row

TRACE = False
LAST_RESULTS = {}

_cache = {}


def _build():
    import concourse.bass as bass  # noqa: F401
    import concourse.mybir as mybir
    import concourse.tile as tile
    from concourse import bacc
    from concourse.masks import make_identity
    from contextlib import ExitStack

    fp32 = mybir.dt.float32
    bf16 = mybir.dt.bfloat16
    RELU = mybir.ActivationFunctionType.Relu
    COPY = mybir.ActivationFunctionType.Copy

    nc = bacc.Bacc("TRN2", target_bir_lowering=False, debug=False,
                   num_devices=NCORES)

    xhi = nc.dram_tensor("xhi", [D, BSH], bf16, kind="ExternalInput").ap()
    xlo = nc.dram_tensor("xlo", [D, BSH], bf16, kind="ExternalInput").ap()
    whi = nc.dram_tensor("whi", [D, H], bf16, kind="ExternalInput").ap()
    wlo = nc.dram_tensor("wlo", [D, H], bf16, kind="ExternalInput").ap()
    bhi = nc.dram_tensor("bhi", [1, H], bf16, kind="ExternalInput").ap()
    blo = nc.dram_tensor("blo", [1, H], bf16, kind="ExternalInput").ap()
    wdecT = nc.dram_tensor("wdecT", [H, O], bf16, kind="ExternalInput").ap()
    bdec = nc.dram_tensor("bdec", [1, O], fp32, kind="ExternalInput").ap()
    enc_out = nc.dram_tensor("enc_sparse", [BSH, H], fp32,
                             kind="ExternalOutput").ap()
    dec_out = nc.dram_tensor("dec", [BSH, O], fp32,
                             kind="ExternalOutput").ap()

    with tile.TileContext(nc) as tc, ExitStack() as ctx:
        const = ctx.enter_context(tc.tile_pool(name="const", bufs=1))
        dram = ctx.enter_context(tc.tile_pool(name="dram", bufs=1,
                                              space="DRAM"))
        xpool = ctx.enter_context(tc.tile_pool(name="xTp", bufs=1))
        wpool = ctx.enter_context(tc.tile_pool(name="wenc", bufs=2))
        cpool = ctx.enter_context(tc.tile_pool(name="encch", bufs=2))
        epool = ctx.enter_context(tc.tile_pool(name="encrow", bufs=2))
        candpool = ctx.enter_context(tc.tile_pool(name="cand", bufs=1))
        etpool = ctx.enter_context(tc.tile_pool(name="encT", bufs=1))
        mpool = ctx.enter_context(tc.tile_pool(name="m8", bufs=8))
        wdpool = ctx.enter_context(tc.tile_pool(name="wdec", bufs=3))
        ecpool = ctx.enter_context(tc.tile_pool(name="encTc", bufs=3))
        dpool = ctx.enter_context(tc.tile_pool(name="decout", bufs=1))
        psA = ctx.enter_context(tc.tile_pool(name="psA", bufs=2,
                                             space="PSUM"))
        psT = ctx.enter_context(tc.tile_pool(name="psT", bufs=2,
                                             space="PSUM"))
        psC = ctx.enter_context(tc.tile_pool(name="psC", bufs=4,
                                             space="PSUM"))

        ident = const.tile([128, 128], fp32)
        make_identity(nc, ident)
        ones_bf = const.tile([1, 128], bf16)
        nc.vector.memset(ones_bf, 1.0)
        ones_f32 = const.tile([1, 128], fp32)
        nc.vector.memset(ones_f32, 1.0)
        bdec_sb = const.tile([1, O], fp32)
        nc.sync.dma_start(out=bdec_sb, in_=bdec)

        enc_dram = [dram.tile([GB, H], fp32, name=f"enc_dram{g}")
                    for g in range(NG)]
        encT_dram = [dram.tile([H, GB], bf16, name=f"encT_dram{g}")
                     for g in range(NG)]

        for g in range(NG):
            gc = slice(g * GB, (g + 1) * GB)  # this group's batch columns
            # ---------------- Phase A(g): encoder ----------------
            xhi_sb = xpool.tile([128, NDC, GB], bf16, tag="xhi",
                                name=f"xhi{g}")
            nc.sync.dma_start(
                out=xhi_sb,
                in_=xhi[:, gc].rearrange("(j p) b -> p j b", p=128))
            xlo_sb = xpool.tile([128, NDC, GB], bf16, tag="xlo",
                                name=f"xlo{g}")
            nc.sync.dma_start(
                out=xlo_sb,
                in_=xlo[:, gc].rearrange("(j p) b -> p j b", p=128))
            for c in range(NHC):
                hs = slice(c * HCH, (c + 1) * HCH)
                whi_sb = wpool.tile([128, NDC, HCH], bf16, tag="whi",
                                    name=f"whi{g}_{c}")
                nc.sync.dma_start(
                    out=whi_sb,
                    in_=whi[:, hs].rearrange("(j p) h -> p j h", p=128))
                wlo_sb = wpool.tile([128, NDC, HCH], bf16, tag="wlo",
                                    name=f"wlo{g}_{c}")
                nc.sync.dma_start(
                    out=wlo_sb,
                    in_=wlo[:, hs].rearrange("(j p) h -> p j h", p=128))
                bhi_sb = wpool.tile([1, HCH], bf16, tag="bhi",
                                    name=f"bhi{g}_{c}")
                nc.sync.dma_start(out=bhi_sb, in_=bhi[:, hs])
                blo_sb = wpool.tile([1, HCH], bf16, tag="blo",
                                    name=f"blo{g}_{c}")
                nc.sync.dma_start(out=blo_sb, in_=blo[:, hs])
                for t in range(NBT):
                    ts_ = slice(t * 128, (t + 1) * 128)
                    ps = psA.tile([128, HCH], fp32, tag="psa",
                                  name=f"psa{g}_{c}_{t}")
                    for d in range(NDC):
                        nc.tensor.matmul(ps, lhsT=xhi_sb[:, d, ts_],
                                         rhs=whi_sb[:, d, :],
                                         start=(d == 0), stop=False)
                        nc.tensor.matmul(ps, lhsT=xhi_sb[:, d, ts_],
                                         rhs=wlo_sb[:, d, :],
                                         start=False, stop=False)
                        nc.tensor.matmul(ps, lhsT=xlo_sb[:, d, ts_],
                                         rhs=whi_sb[:, d, :],
                                         start=False, stop=False)
                    nc.tensor.matmul(ps, lhsT=ones_bf, rhs=bhi_sb,
                                     start=False, stop=False)
                    nc.tensor.matmul(ps, lhsT=ones_bf, rhs=blo_sb,
                                     start=False, stop=True)
                    ch = cpool.tile([128, HCH], fp32, tag="encch",
                                    name=f"ch{g}_{c}_{t}")
                    nc.scalar.activation(ch, ps, RELU)
                    nc.scalar.dma_start(out=enc_dram[g][ts_, hs], in_=ch)

            # ---------------- Phase B(g): top-k + transpose ----------
            for t in range(NBT):
                ts_ = slice(t * 128, (t + 1) * 128)
                enc = epool.tile([128, H], fp32, tag="enc",
                                 name=f"enc{g}_{t}")
                for q in range(4):
                    qs = slice(q * (H // 4), (q + 1) * (H // 4))
                    eng = (nc.sync, nc.gpsimd, nc.scalar, nc.sync)[q]
                    eng.dma_start(out=enc[:, qs], in_=enc_dram[g][ts_, qs])
                # segmented top-8 candidates: top-32 of the row is within
                # the union of per-256-segment top-8s (whp)
                cand = candpool.tile([128, NSEG * 8], fp32, tag="cand",
                                     name=f"cand{g}_{t}")
                for sgi in range(NSEG):
                    nc.vector.max(
                        out=cand[:, sgi * 8:(sgi + 1) * 8],
                        in_=enc[:, sgi * SEG:(sgi + 1) * SEG])
                mlast = None
                for r in range(K // 8):
                    m = mpool.tile([128, 8], fp32, tag="m8",
                                   name=f"m{g}_{t}_{r}")
                    nc.vector.max(out=m, in_=cand)
                    if r < K // 8 - 1:
                        nc.vector.match_replace(out=cand, in_to_replace=m,
                                                in_values=cand,
                                                imm_value=0.0)
                    mlast = m
                # v32 = 32nd largest; keep values >= v32 (single fused pass)
                nc.vector.scalar_tensor_tensor(
                    out=enc, in0=enc, scalar=mlast[:, 7:8], in1=enc,
                    op0=mybir.AluOpType.is_ge, op1=mybir.AluOpType.mult)
                for q in range(4):
                    qs = slice(q * (H // 4), (q + 1) * (H // 4))
                    eng = (nc.scalar, nc.sync, nc.gpsimd, nc.scalar)[q]
                    eng.dma_start(
                        out=enc_out[g * GB + t * 128:g * GB + (t + 1) * 128,
                                    qs],
                        in_=enc[:, qs])
                # transpose masked tile -> bf16 encT (quarters)
                for q in range(4):
                    encT_sb = etpool.tile([128, 32, 128], bf16, tag="encT",
                                          name=f"encT{g}_{t}_{q}")
                    for j in range(32):
                        jj = q * 32 + j
                        pst = psT.tile([128, 128], fp32, tag="pst",
                                       name=f"pst{g}_{t}_{q}_{j}")
                        nc.tensor.transpose(
                            pst, enc[:, jj * 128:(jj + 1) * 128], ident)
                        nc.scalar.activation(encT_sb[:, j, :], pst, COPY)
                    nc.sync.dma_start(
                        out=encT_dram[g].rearrange(
                            "(j p) b -> p j b",
                            p=128)[:, q * 32:(q + 1) * 32, ts_],
                        in_=encT_sb)

            # ---------------- Phase C(g): decoder ----------------
            for oh in range(2):
                os_ = slice(oh * 512, (oh + 1) * 512)
                pss = [psC.tile([128, 512], fp32, tag="psdec",
                                name=f"psdec{g}_{oh}_{i}")
                       for i in range(NBT)]
                for c in range(H // 128):
                    cs = slice(c * 128, (c + 1) * 128)
                    wd = wdpool.tile([128, 512], bf16, tag="wd",
                                     name=f"wd{g}_{oh}_{c}")
                    nc.sync.dma_start(out=wd, in_=wdecT[cs, os_])
                    et = ecpool.tile([128, GB], bf16, tag="et",
                                     name=f"et{g}_{oh}_{c}")
                    nc.sync.dma_start(out=et, in_=encT_dram[g][cs, :])
                    for t in range(NBT):
                        nc.tensor.matmul(
                            pss[t], lhsT=et[:, t * 128:(t + 1) * 128],
                            rhs=wd, start=(c == 0), stop=False)
                for t in range(NBT):
                    nc.tensor.matmul(pss[t], lhsT=ones_f32,
                                     rhs=bdec_sb[:, os_],
                                     start=False, stop=True)
                    do = dpool.tile([128, 512], fp32, tag="do",
                                    name=f"do{g}_{oh}_{t}")
                    nc.scalar.activation(do, pss[t], COPY)
                    nc.sync.dma_start(
                        out=dec_out[g * GB + t * 128:g * GB + (t + 1) * 128,
                                    os_],
                        in_=do)

    nc.compile()
    return nc


def _split_bf16(a):
    hi = a.astype(ml_dtypes.bfloat16)
    lo = (a - hi.astype(np.float32)).astype(ml_dtypes.bfloat16)
    return hi, lo


def kernel(x, W_enc, b_enc, W_dec, b_dec, topk):
    assert int(topk) == K
    from concourse.bass_utils import run_bass_kernel_spmd

    x = np.asarray(x, dtype=np.float32)
    W_enc = np.asarray(W_enc, dtype=np.float32)
    b_enc = np.asarray(b_enc, dtype=np.float32)
    W_dec = np.asarray(W_dec, dtype=np.float32)
    b_dec = np.asarray(b_dec, dtype=np.float32)

    if "nc" not in _cache:
        _cache["nc"] = _build()
    nc = _cache["nc"]

    xT = np.ascontiguousarray(x.T)  # [D, B]
    xT_hi, xT_lo = _split_bf16(xT)
    wencT = np.ascontiguousarray(W_enc.T)  # [D, H]
    w_hi, w_lo = _split_bf16(wencT)
    b_hi, b_lo = _split_bf16(b_enc.reshape(1, H))
    wdecT = np.ascontiguousarray(W_dec.T).astype(ml_dtypes.bfloat16)
    bdec = np.ascontiguousarray(b_dec.reshape(1, O))

    in_maps = []
    for c in range(NCORES):
        cs = slice(c * BSH, (c + 1) * BSH)
        in_maps.append({
            "xhi": np.ascontiguousarray(xT_hi[:, cs]),
            "xlo": np.ascontiguousarray(xT_lo[:, cs]),
            "whi": w_hi,
            "wlo": w_lo,
            "bhi": b_hi,
            "blo": b_lo,
            "wdecT": wdecT,
            "bdec": bdec,
        })

    res = run_bass_kernel_spmd(nc, in_maps, core_ids=list(range(NCORES)),
                               trace=TRACE)
    LAST_RESULTS["exec_time_ns"] = res.exec_time_ns
    LAST_RESULTS["profile_json"] = res.profile_json

    enc_sparse = np.concatenate([res.results[c]["enc_sparse"]
                                 for c in range(NCORES)], axis=0)
    dec = np.concatenate([res.results[c]["dec"]
                          for c in range(NCORES)], axis=0)
    return enc_sparse.astype(np.float32), dec.astype(np.float32)


# revision 15
# speedup vs baseline: 1.6429x; 1.0649x over previous
"""Trainium2 Bass kernel for sparse autoencoder (topk masking).

  encoder:  pre = x @ W_enc.T + b_enc ; enc = relu(pre)
  topk:     per-row top-32 of enc kept, rest zeroed  -> encoded_sparse
  decoder:  dec = encoded_sparse @ W_dec.T + b_dec
  returns (encoded_sparse, dec)

Sharding: pure data-parallel over the batch dim across 8 NeuronCores
(1024 rows per core).  No collectives needed.

v6 pipeline per core (2 batch groups of 512 rows for cross-phase
overlap):
  Phase A(g): bf16 hi/lo-split matmuls (fp32-accurate), fused ReLU on
      ScalarE, park dense enc rows in DRAM.  While each relu chunk is
      still in SBUF, VectorE computes per-128-column-segment top-8
      candidates (no extra HBM traffic).
  Phase B(g): per row-tile: 7 small ops on the 1024-wide candidate
      array give the exact 32nd-largest value v32; then a streaming
      quarter-pipeline re-reads enc, applies the fused
      (enc >= v32) * enc mask in one VectorE pass, writes
      encoded_sparse, and PE-transposes to bf16 encT for the decoder.
  Phase C(g): dense bf16 decode, W_decT streamed, PSUM accumulation
      over 128 h-chunks, bias via K=1 matmul.
"""

import sys

sys.path.insert(0, "/opt/trn_rl_repo")

import numpy as np
import ml_dtypes

B, D, H, O, K = 8192, 1024, 16384, 1024, 32
NCORES = 8
BSH = B // NCORES  # 1024 rows per core
NG = 2             # batch groups per core
GB = BSH // NG     # 512 rows per group
NBT = GB // 128    # 4 row-tiles per group
HCH = 512          # encoder h-chunk
NHC = H // HCH     # 32
NDC = D // 128     # 8 contraction chunks
SEG = 128          # top-k candidate segment width
NSEG = H // SEG    # 128 segments -> 1024 candidates/row
HQ = H // 4        # phase-B quarter width (4096)

TRACE = False
LAST_RESULTS = {}

_cache = {}


def _build():
    import concourse.bass as bass  # noqa: F401
    import concourse.mybir as mybir
    import concourse.tile as tile
    from concourse import bacc
    from concourse.masks import make_identity
    from contextlib import ExitStack

    fp32 = mybir.dt.float32
    bf16 = mybir.dt.bfloat16
    RELU = mybir.ActivationFunctionType.Relu
    COPY = mybir.ActivationFunctionType.Copy

    nc = bacc.Bacc("TRN2", target_bir_lowering=False, debug=False,
                   num_devices=NCORES)

    xhi = nc.dram_tensor("xhi", [D, BSH], bf16, kind="ExternalInput").ap()
    xlo = nc.dram_tensor("xlo", [D, BSH], bf16, kind="ExternalInput").ap()
    whi = nc.dram_tensor("whi", [D, H], bf16, kind="ExternalInput").ap()
    wlo = nc.dram_tensor("wlo", [D, H], bf16, kind="ExternalInput").ap()
    bstack = nc.dram_tensor("bstack", [2, H], bf16, kind="ExternalInput").ap()
    wdecT = nc.dram_tensor("wdecT", [H, O], bf16, kind="ExternalInput").ap()
    bdec = nc.dram_tensor("bdec", [1, O], fp32, kind="ExternalInput").ap()
    enc_out = nc.dram_tensor("enc_sparse", [BSH, H], fp32,
                             kind="ExternalOutput").ap()
    dec_out = nc.dram_tensor("dec", [BSH, O], fp32,
                             kind="ExternalOutput").ap()

    with tile.TileContext(nc) as tc, ExitStack() as ctx:
        const = ctx.enter_context(tc.tile_pool(name="const", bufs=1))
        dram = ctx.enter_context(tc.tile_pool(name="dram", bufs=1,
                                              space="DRAM"))
        xpool = ctx.enter_context(tc.tile_pool(name="xTp", bufs=1))
        wpool = ctx.enter_context(tc.tile_pool(name="wenc", bufs=2))
        cpool = ctx.enter_context(tc.tile_pool(name="encch", bufs=3))
        candpool = ctx.enter_context(tc.tile_pool(name="cand", bufs=8))
        eqpool = ctx.enter_context(tc.tile_pool(name="encq", bufs=3))
        etpool = ctx.enter_context(tc.tile_pool(name="encT", bufs=2))
        mpool = ctx.enter_context(tc.tile_pool(name="m8", bufs=8))
        wdpool = ctx.enter_context(tc.tile_pool(name="wdec", bufs=3))
        ecpool = ctx.enter_context(tc.tile_pool(name="encTc", bufs=3))
        dpool = ctx.enter_context(tc.tile_pool(name="decout", bufs=2))
        psA = ctx.enter_context(tc.tile_pool(name="psA", bufs=2,
                                             space="PSUM"))
        psT = ctx.enter_context(tc.tile_pool(name="psT", bufs=2,
                                             space="PSUM"))
        psC = ctx.enter_context(tc.tile_pool(name="psC", bufs=4,
                                             space="PSUM"))

        ident = const.tile([128, 128], fp32)
        make_identity(nc, ident)
        ones2 = const.tile([2, 128], bf16)
        nc.vector.memset(ones2, 1.0)
        ones_f32 = const.tile([1, 128], fp32)
        nc.vector.memset(ones_f32, 1.0)
        bdec_sb = const.tile([1, O], fp32)
        nc.sync.dma_start(out=bdec_sb, in_=bdec)

        enc_dram = [dram.tile([GB, H], fp32, name=f"enc_dram{g}")
                    for g in range(NG)]
        encT_dram = [dram.tile([H, GB], bf16, name=f"encT_dram{g}")
                     for g in range(NG)]

        for g in range(NG):
            gc = slice(g * GB, (g + 1) * GB)  # this group's batch columns
            # ---------------- Phase A(g): encoder + candidates ---------
            xhi_sb = xpool.tile([128, NDC, GB], bf16, tag="xhi",
                                name=f"xhi{g}")
            nc.sync.dma_start(
                out=xhi_sb,
                in_=xhi[:, gc].rearrange("(j p) b -> p j b", p=128))
            xlo_sb = xpool.tile([128, NDC, GB], bf16, tag="xlo",
                                name=f"xlo{g}")
            nc.sync.dma_start(
                out=xlo_sb,
                in_=xlo[:, gc].rearrange("(j p) b -> p j b", p=128))
            cands = [candpool.tile([128, NSEG * 8], fp32, tag="cand",
                                   name=f"cand{g}_{t}")
                     for t in range(NBT)]
            for c in range(NHC):
                hs = slice(c * HCH, (c + 1) * HCH)
                whi_sb = wpool.tile([128, NDC, HCH], bf16, tag="whi",
                                    name=f"whi{g}_{c}")
                nc.sync.dma_start(
                    out=whi_sb,
                    in_=whi[:, hs].rearrange("(j p) h -> p j h", p=128))
                wlo_sb = wpool.tile([128, NDC, HCH], bf16, tag="wlo",
                                    name=f"wlo{g}_{c}")
                nc.sync.dma_start(
                    out=wlo_sb,
                    in_=wlo[:, hs].rearrange("(j p) h -> p j h", p=128))
                bst_sb = wpool.tile([2, HCH], bf16, tag="bst",
                                    name=f"bst{g}_{c}")
                nc.sync.dma_start(out=bst_sb, in_=bstack[:, hs])
                for t in range(NBT):
                    ts_ = slice(t * 128, (t + 1) * 128)
                    ps = psA.tile([128, HCH], fp32, tag="psa",
                                  name=f"psa{g}_{c}_{t}")
                    for d in range(NDC):
                        nc.tensor.matmul(ps, lhsT=xhi_sb[:, d, ts_],
                                         rhs=whi_sb[:, d, :],
                                         start=(d == 0), stop=False)
                        nc.tensor.matmul(ps, lhsT=xhi_sb[:, d, ts_],
                                         rhs=wlo_sb[:, d, :],
                                         start=False, stop=False)
                        nc.tensor.matmul(ps, lhsT=xlo_sb[:, d, ts_],
                                         rhs=whi_sb[:, d, :],
                                         start=False, stop=False)
                    nc.tensor.matmul(ps, lhsT=ones2, rhs=bst_sb,
                                     start=False, stop=True)
                    ch = cpool.tile([128, HCH], fp32, tag="encch",
                                    name=f"ch{g}_{c}_{t}")
                    nc.scalar.activation(ch, ps, RELU)
                    nc.scalar.dma_start(out=enc_dram[g][ts_, hs], in_=ch)
                    # per-128-segment top-8 candidates while chunk is hot
                    for si in range(HCH // SEG):
                        sgi = c * (HCH // SEG) + si
                        nc.vector.max(
                            out=cands[t][:, sgi * 8:(sgi + 1) * 8],
                            in_=ch[:, si * SEG:(si + 1) * SEG])

            # ---------------- Phase B(g): threshold + mask + transpose --
            for t in range(NBT):
                ts_ = slice(t * 128, (t + 1) * 128)
                cand = cands[t]
                mlast = None
                for r in range(K // 8):
                    m = mpool.tile([128, 8], fp32, tag="m8",
                                   name=f"m{g}_{t}_{r}")
                    nc.vector.max(out=m, in_=cand)
                    if r < K // 8 - 1:
                        nc.vector.match_replace(out=cand, in_to_replace=m,
                                                in_values=cand,
                                                imm_value=0.0)
                    mlast = m
                v32 = mlast[:, 7:8]
                for q in range(4):
                    qs = slice(q * HQ, (q + 1) * HQ)
                    eq = eqpool.tile([128, HQ], fp32, tag="eq",
                                     name=f"eq{g}_{t}_{q}")
                    eng = (nc.sync, nc.gpsimd, nc.scalar, nc.sync)[q]
                    eng.dma_start(out=eq, in_=enc_dram[g][ts_, qs])
                    nc.vector.scalar_tensor_tensor(
                        out=eq, in0=eq, scalar=v32, in1=eq,
                        op0=mybir.AluOpType.is_ge, op1=mybir.AluOpType.mult)
                    eng2 = (nc.gpsimd, nc.scalar, nc.sync, nc.gpsimd)[q]
                    eng2.dma_start(
                        out=enc_out[g * GB + t * 128:g * GB + (t + 1) * 128,
                                    qs],
                        in_=eq)
                    encT_sb = etpool.tile([128, 32, 128], bf16, tag="encT",
                                          name=f"encT{g}_{t}_{q}")
                    for j in range(32):
                        pst = psT.tile([128, 128], fp32, tag="pst",
                                       name=f"pst{g}_{t}_{q}_{j}")
                        nc.tensor.transpose(
                            pst, eq[:, j * 128:(j + 1) * 128], ident)
                        nc.scalar.activation(encT_sb[:, j, :], pst, COPY)
                    nc.sync.dma_start(
                        out=encT_dram[g].rearrange(
                            "(j p) b -> p j b",
                            p=128)[:, q * 32:(q + 1) * 32, ts_],
                        in_=encT_sb)

            # ---------------- Phase C(g): decoder ----------------
            for oh in range(2):
                os_ = slice(oh * 512, (oh + 1) * 512)
                pss = [psC.tile([128, 512], fp32, tag="psdec",
                                name=f"psdec{g}_{oh}_{i}")
                       for i in range(NBT)]
                for c in range(H // 128):
                    cs = slice(c * 128, (c + 1) * 128)
                    wd = wdpool.tile([128, 512], bf16, tag="wd",
                                     name=f"wd{g}_{oh}_{c}")
                    nc.sync.dma_start(out=wd, in_=wdecT[cs, os_])
                    et = ecpool.tile([128, GB], bf16, tag="et",
                                     name=f"et{g}_{oh}_{c}")
                    nc.sync.dma_start(out=et, in_=encT_dram[g][cs, :])
                    for t in range(NBT):
                        nc.tensor.matmul(
                            pss[t], lhsT=et[:, t * 128:(t + 1) * 128],
                            rhs=wd, start=(c == 0), stop=False)
                for t in range(NBT):
                    nc.tensor.matmul(pss[t], lhsT=ones_f32,
                                     rhs=bdec_sb[:, os_],
                                     start=False, stop=True)
                    do = dpool.tile([128, 512], fp32, tag="do",
                                    name=f"do{g}_{oh}_{t}")
                    nc.scalar.activation(do, pss[t], COPY)
                    nc.scalar.dma_start(
                        out=dec_out[g * GB + t * 128:g * GB + (t + 1) * 128,
                                    os_],
                        in_=do)

    nc.compile()
    return nc


def _split_bf16(a):
    hi = a.astype(ml_dtypes.bfloat16)
    lo = (a - hi.astype(np.float32)).astype(ml_dtypes.bfloat16)
    return hi, lo


def kernel(x, W_enc, b_enc, W_dec, b_dec, topk):
    assert int(topk) == K
    from concourse.bass_utils import run_bass_kernel_spmd

    x = np.asarray(x, dtype=np.float32)
    W_enc = np.asarray(W_enc, dtype=np.float32)
    b_enc = np.asarray(b_enc, dtype=np.float32)
    W_dec = np.asarray(W_dec, dtype=np.float32)
    b_dec = np.asarray(b_dec, dtype=np.float32)

    if "nc" not in _cache:
        _cache["nc"] = _build()
    nc = _cache["nc"]

    xT = np.ascontiguousarray(x.T)  # [D, B]
    xT_hi, xT_lo = _split_bf16(xT)
    wencT = np.ascontiguousarray(W_enc.T)  # [D, H]
    w_hi, w_lo = _split_bf16(wencT)
    b_hi, b_lo = _split_bf16(b_enc.reshape(1, H))
    bstack = np.ascontiguousarray(np.concatenate([b_hi, b_lo], axis=0))
    wdecT = np.ascontiguousarray(W_dec.T).astype(ml_dtypes.bfloat16)
    bdec = np.ascontiguousarray(b_dec.reshape(1, O))

    in_maps = []
    for c in range(NCORES):
        cs = slice(c * BSH, (c + 1) * BSH)
        in_maps.append({
            "xhi": np.ascontiguousarray(xT_hi[:, cs]),
            "xlo": np.ascontiguousarray(xT_lo[:, cs]),
            "whi": w_hi,
            "wlo": w_lo,
            "bstack": bstack,
            "wdecT": wdecT,
            "bdec": bdec,
        })

    res = run_bass_kernel_spmd(nc, in_maps, core_ids=list(range(NCORES)),
                               trace=TRACE)
    LAST_RESULTS["exec_time_ns"] = res.exec_time_ns
    LAST_RESULTS["profile_json"] = res.profile_json

    enc_sparse = np.concatenate([res.results[c]["enc_sparse"]
                                 for c in range(NCORES)], axis=0)
    dec = np.concatenate([res.results[c]["dec"]
                          for c in range(NCORES)], axis=0)
    return enc_sparse.astype(np.float32), dec.astype(np.float32)


# revision 16
# speedup vs baseline: 1.6532x; 1.0063x over previous
"""Trainium2 Bass kernel for sparse autoencoder (topk masking).

  encoder:  pre = x @ W_enc.T + b_enc ; enc = relu(pre)
  topk:     per-row top-32 of enc kept, rest zeroed  -> encoded_sparse
  decoder:  dec = encoded_sparse @ W_dec.T + b_dec
  returns (encoded_sparse, dec)

Sharding: pure data-parallel over the batch dim across 8 NeuronCores
(1024 rows per core).  No collectives needed.

v6 pipeline per core (2 batch groups of 512 rows for cross-phase
overlap):
  Phase A(g): bf16 hi/lo-split matmuls (fp32-accurate), fused ReLU on
      ScalarE, park dense enc rows in DRAM.  While each relu chunk is
      still in SBUF, VectorE computes per-128-column-segment top-8
      candidates (no extra HBM traffic).
  Phase B(g): per row-tile: 7 small ops on the 1024-wide candidate
      array give the exact 32nd-largest value v32; then a streaming
      quarter-pipeline re-reads enc, applies the fused
      (enc >= v32) * enc mask in one VectorE pass, writes
      encoded_sparse, and PE-transposes to bf16 encT for the decoder.
  Phase C(g): dense bf16 decode, W_decT streamed, PSUM accumulation
      over 128 h-chunks, bias via K=1 matmul.
"""

import sys

sys.path.insert(0, "/opt/trn_rl_repo")

import numpy as np
import ml_dtypes

B, D, H, O, K = 8192, 1024, 16384, 1024, 32
NCORES = 8
BSH = B // NCORES  # 1024 rows per core
NG = 2             # batch groups per core
GB = BSH // NG     # 512 rows per group
NBT = GB // 128    # 4 row-tiles per group
HCH = 512          # encoder h-chunk
NHC = H // HCH     # 32
NDC = D // 128     # 8 contraction chunks
SEG = 128          # top-k candidate segment width
NSEG = H // SEG    # 128 segments -> 1024 candidates/row
HQ = H // 4        # phase-B quarter width (4096)

TRACE = False
LAST_RESULTS = {}

_cache = {}


def _build():
    import concourse.bass as bass  # noqa: F401
    import concourse.mybir as mybir
    import concourse.tile as tile
    from concourse import bacc
    from concourse.masks import make_identity
    from contextlib import ExitStack

    fp32 = mybir.dt.float32
    bf16 = mybir.dt.bfloat16
    RELU = mybir.ActivationFunctionType.Relu
    COPY = mybir.ActivationFunctionType.Copy

    nc = bacc.Bacc("TRN2", target_bir_lowering=False, debug=False,
                   num_devices=NCORES)

    xhi = nc.dram_tensor("xhi", [D, BSH], bf16, kind="ExternalInput").ap()
    xlo = nc.dram_tensor("xlo", [D, BSH], bf16, kind="ExternalInput").ap()
    whi = nc.dram_tensor("whi", [D, H], bf16, kind="ExternalInput").ap()
    wlo = nc.dram_tensor("wlo", [D, H], bf16, kind="ExternalInput").ap()
    bstack = nc.dram_tensor("bstack", [2, H], bf16, kind="ExternalInput").ap()
    wdecT = nc.dram_tensor("wdecT", [H, O], bf16, kind="ExternalInput").ap()
    bdec = nc.dram_tensor("bdec", [1, O], fp32, kind="ExternalInput").ap()
    enc_out = nc.dram_tensor("enc_sparse", [BSH, H], fp32,
                             kind="ExternalOutput").ap()
    dec_out = nc.dram_tensor("dec", [BSH, O], fp32,
                             kind="ExternalOutput").ap()

    with tile.TileContext(nc) as tc, ExitStack() as ctx:
        const = ctx.enter_context(tc.tile_pool(name="const", bufs=1))
        dram = ctx.enter_context(tc.tile_pool(name="dram", bufs=1,
                                              space="DRAM"))
        xpool = ctx.enter_context(tc.tile_pool(name="xTp", bufs=1))
        wpool = ctx.enter_context(tc.tile_pool(name="wenc", bufs=2))
        cpool = ctx.enter_context(tc.tile_pool(name="encch", bufs=3))
        candpool = ctx.enter_context(tc.tile_pool(name="cand", bufs=8))
        eqpool = ctx.enter_context(tc.tile_pool(name="encq", bufs=3))
        etpool = ctx.enter_context(tc.tile_pool(name="encT", bufs=2))
        mpool = ctx.enter_context(tc.tile_pool(name="m8", bufs=8))
        wdpool = ctx.enter_context(tc.tile_pool(name="wdec", bufs=3))
        ecpool = ctx.enter_context(tc.tile_pool(name="encTc", bufs=3))
        dpool = ctx.enter_context(tc.tile_pool(name="decout", bufs=2))
        psA = ctx.enter_context(tc.tile_pool(name="psA", bufs=2,
                                             space="PSUM"))
        psT = ctx.enter_context(tc.tile_pool(name="psT", bufs=2,
                                             space="PSUM"))
        psC = ctx.enter_context(tc.tile_pool(name="psC", bufs=4,
                                             space="PSUM"))

        ident = const.tile([128, 128], fp32)
        make_identity(nc, ident)
        ones2 = const.tile([2, 128], bf16)
        nc.vector.memset(ones2, 1.0)
        ones_f32 = const.tile([1, 128], fp32)
        nc.vector.memset(ones_f32, 1.0)
        bdec_sb = const.tile([1, O], fp32)
        nc.sync.dma_start(out=bdec_sb, in_=bdec)

        enc_dram = [dram.tile([GB, H], fp32, name=f"enc_dram{g}")
                    for g in range(NG)]
        encT_dram = [[dram.tile([H // 4, GB], bf16,
                                name=f"encT_dram{g}_{q}")
                      for q in range(4)] for g in range(NG)]

        for g in range(NG):
            gc = slice(g * GB, (g + 1) * GB)  # this group's batch columns
            # ---------------- Phase A(g): encoder + candidates ---------
            xhi_sb = xpool.tile([128, NDC, GB], bf16, tag="xhi",
                                name=f"xhi{g}")
            nc.sync.dma_start(
                out=xhi_sb,
                in_=xhi[:, gc].rearrange("(j p) b -> p j b", p=128))
            xlo_sb = xpool.tile([128, NDC, GB], bf16, tag="xlo",
                                name=f"xlo{g}")
            nc.sync.dma_start(
                out=xlo_sb,
                in_=xlo[:, gc].rearrange("(j p) b -> p j b", p=128))
            cands = [candpool.tile([128, NSEG * 8], fp32, tag="cand",
                                   name=f"cand{g}_{t}")
                     for t in range(NBT)]
            for c in range(NHC):
                hs = slice(c * HCH, (c + 1) * HCH)
                whi_sb = wpool.tile([128, NDC, HCH], bf16, tag="whi",
                                    name=f"whi{g}_{c}")
                nc.sync.dma_start(
                    out=whi_sb,
                    in_=whi[:, hs].rearrange("(j p) h -> p j h", p=128))
                wlo_sb = wpool.tile([128, NDC, HCH], bf16, tag="wlo",
                                    name=f"wlo{g}_{c}")
                nc.sync.dma_start(
                    out=wlo_sb,
                    in_=wlo[:, hs].rearrange("(j p) h -> p j h", p=128))
                bst_sb = wpool.tile([2, HCH], bf16, tag="bst",
                                    name=f"bst{g}_{c}")
                nc.sync.dma_start(out=bst_sb, in_=bstack[:, hs])
                for t in range(NBT):
                    ts_ = slice(t * 128, (t + 1) * 128)
                    ps = psA.tile([128, HCH], fp32, tag="psa",
                                  name=f"psa{g}_{c}_{t}")
                    for d in range(NDC):
                        nc.tensor.matmul(ps, lhsT=xhi_sb[:, d, ts_],
                                         rhs=whi_sb[:, d, :],
                                         start=(d == 0), stop=False)
                        nc.tensor.matmul(ps, lhsT=xhi_sb[:, d, ts_],
                                         rhs=wlo_sb[:, d, :],
                                         start=False, stop=False)
                        nc.tensor.matmul(ps, lhsT=xlo_sb[:, d, ts_],
                                         rhs=whi_sb[:, d, :],
                                         start=False, stop=False)
                    nc.tensor.matmul(ps, lhsT=ones2, rhs=bst_sb,
                                     start=False, stop=True)
                    ch = cpool.tile([128, HCH], fp32, tag="encch",
                                    name=f"ch{g}_{c}_{t}")
                    nc.scalar.activation(ch, ps, RELU)
                    nc.scalar.dma_start(out=enc_dram[g][ts_, hs], in_=ch)
                    # per-128-segment top-8 candidates while chunk is hot
                    for si in range(HCH // SEG):
                        sgi = c * (HCH // SEG) + si
                        nc.vector.max(
                            out=cands[t][:, sgi * 8:(sgi + 1) * 8],
                            in_=ch[:, si * SEG:(si + 1) * SEG])

            # ---------------- Phase B(g): threshold + mask + transpose --
            for t in range(NBT):
                ts_ = slice(t * 128, (t + 1) * 128)
                cand = cands[t]
                mlast = None
                for r in range(K // 8):
                    m = mpool.tile([128, 8], fp32, tag="m8",
                                   name=f"m{g}_{t}_{r}")
                    nc.vector.max(out=m, in_=cand)
                    if r < K // 8 - 1:
                        nc.vector.match_replace(out=cand, in_to_replace=m,
                                                in_values=cand,
                                                imm_value=0.0)
                    mlast = m
                v32 = mlast[:, 7:8]
                for q in range(4):
                    qs = slice(q * HQ, (q + 1) * HQ)
                    eq = eqpool.tile([128, HQ], fp32, tag="eq",
                                     name=f"eq{g}_{t}_{q}")
                    eng = (nc.sync, nc.gpsimd, nc.scalar, nc.sync)[q]
                    eng.dma_start(out=eq, in_=enc_dram[g][ts_, qs])
                    nc.vector.scalar_tensor_tensor(
                        out=eq, in0=eq, scalar=v32, in1=eq,
                        op0=mybir.AluOpType.is_ge, op1=mybir.AluOpType.mult)
                    eng2 = (nc.gpsimd, nc.scalar, nc.sync, nc.gpsimd)[q]
                    eng2.dma_start(
                        out=enc_out[g * GB + t * 128:g * GB + (t + 1) * 128,
                                    qs],
                        in_=eq)
                    encT_sb = etpool.tile([128, 32, 128], bf16, tag="encT",
                                          name=f"encT{g}_{t}_{q}")
                    for j in range(32):
                        pst = psT.tile([128, 128], fp32, tag="pst",
                                       name=f"pst{g}_{t}_{q}_{j}")
                        nc.tensor.transpose(
                            pst, eq[:, j * 128:(j + 1) * 128], ident)
                        nc.scalar.activation(encT_sb[:, j, :], pst, COPY)
                    nc.sync.dma_start(
                        out=encT_dram[g][q].rearrange(
                            "(j p) b -> p j b", p=128)[:, :, ts_],
                        in_=encT_sb)

            # ---------------- Phase C(g): decoder ----------------
            for oh in range(2):
                os_ = slice(oh * 512, (oh + 1) * 512)
                pss = [psC.tile([128, 512], fp32, tag="psdec",
                                name=f"psdec{g}_{oh}_{i}")
                       for i in range(NBT)]
                for c in range(H // 128):
                    cs = slice(c * 128, (c + 1) * 128)
                    wd = wdpool.tile([128, 512], bf16, tag="wd",
                                     name=f"wd{g}_{oh}_{c}")
                    nc.sync.dma_start(out=wd, in_=wdecT[cs, os_])
                    et = ecpool.tile([128, GB], bf16, tag="et",
                                     name=f"et{g}_{oh}_{c}")
                    nc.sync.dma_start(
                        out=et,
                        in_=encT_dram[g][c // 32][(c % 32) * 128:
                                                  (c % 32 + 1) * 128, :])
                    for t in range(NBT):
                        nc.tensor.matmul(
                            pss[t], lhsT=et[:, t * 128:(t + 1) * 128],
                            rhs=wd, start=(c == 0), stop=False)
                for t in range(NBT):
                    nc.tensor.matmul(pss[t], lhsT=ones_f32,
                                     rhs=bdec_sb[:, os_],
                                     start=False, stop=True)
                    do = dpool.tile([128, 512], fp32, tag="do",
                                    name=f"do{g}_{oh}_{t}")
                    nc.scalar.activation(do, pss[t], COPY)
                    nc.scalar.dma_start(
                        out=dec_out[g * GB + t * 128:g * GB + (t + 1) * 128,
                                    os_],
                        in_=do)

    nc.compile()
    return nc


def _split_bf16(a):
    hi = a.astype(ml_dtypes.bfloat16)
    lo = (a - hi.astype(np.float32)).astype(ml_dtypes.bfloat16)
    return hi, lo


def kernel(x, W_enc, b_enc, W_dec, b_dec, topk):
    assert int(topk) == K
    from concourse.bass_utils import run_bass_kernel_spmd

    x = np.asarray(x, dtype=np.float32)
    W_enc = np.asarray(W_enc, dtype=np.float32)
    b_enc = np.asarray(b_enc, dtype=np.float32)
    W_dec = np.asarray(W_dec, dtype=np.float32)
    b_dec = np.asarray(b_dec, dtype=np.float32)

    if "nc" not in _cache:
        _cache["nc"] = _build()
    nc = _cache["nc"]

    xT = np.ascontiguousarray(x.T)  # [D, B]
    xT_hi, xT_lo = _split_bf16(xT)
    wencT = np.ascontiguousarray(W_enc.T)  # [D, H]
    w_hi, w_lo = _split_bf16(wencT)
    b_hi, b_lo = _split_bf16(b_enc.reshape(1, H))
    bstack = np.ascontiguousarray(np.concatenate([b_hi, b_lo], axis=0))
    wdecT = np.ascontiguousarray(W_dec.T).astype(ml_dtypes.bfloat16)
    bdec = np.ascontiguousarray(b_dec.reshape(1, O))

    in_maps = []
    for c in range(NCORES):
        cs = slice(c * BSH, (c + 1) * BSH)
        in_maps.append({
            "xhi": np.ascontiguousarray(xT_hi[:, cs]),
            "xlo": np.ascontiguousarray(xT_lo[:, cs]),
            "whi": w_hi,
            "wlo": w_lo,
            "bstack": bstack,
            "wdecT": wdecT,
            "bdec": bdec,
        })

    res = run_bass_kernel_spmd(nc, in_maps, core_ids=list(range(NCORES)),
                               trace=TRACE)
    LAST_RESULTS["exec_time_ns"] = res.exec_time_ns
    LAST_RESULTS["profile_json"] = res.profile_json

    enc_sparse = np.concatenate([res.results[c]["enc_sparse"]
                                 for c in range(NCORES)], axis=0)
    dec = np.concatenate([res.results[c]["dec"]
                          for c in range(NCORES)], axis=0)
    return enc_sparse.astype(np.float32), dec.astype(np.float32)
